# revision 91
# baseline (speedup 1.0000x reference)
"""GQA attention kernel for Trainium2, tensor-parallel over heads on 8 cores.

Problem: B=1, T=2048, EMB=4096, H=32 query heads, G=8 KV groups, D=128.
Reference: q/k/v projections -> per-head RMS norm (q,k) -> RoPE (q,k) ->
causal GQA attention -> out projection.

Sharding: core c owns query heads [4c, 4c+4) and KV group c.  Each core
computes a partial output for its heads; host sums the 8 partials.

v2 design (430us baseline -> ~428us measured, run noise +-4us):
  - DMA posts in need-order (wq e-block 0, xstrip0, ...) ahead of the
    engine-const setup: first matmul at ~14us instead of ~23us.
  - Single global PE stream: every strip's e-loop carries "fill slots"
    that interleave (a) the previous strips' PE transposes, (b) attention
    blocks of ready query slices, (c) out-projection units.  HAM stays at
    K=8/8 from ~19us to ~370us and the A+mix region is ~98.6% PE-busy.
  - Projection results are moved PSUM->SBUF by the bias-add itself, so
    accumulation banks free within ~0.5us of each stop.
  - PSUM zero-region discipline (hard-won): a start=True matmul marks its
    WHOLE 2KB bank pending-zero, while Tile only sequences instructions
    whose byte ranges overlap.  Every bank's consecutive tenants must
    therefore have overlapping ranges, and a ring slot may only be
    re-claimed after its previous consumer has been EMITTED.  Banks:
    S-ring x3, misc ring x2 (out-proj/den/transposes), q-pass x1,
    ctx x1, kv x1.
  - Tail (B(3)+C(2..3) after the last e-loop) runs TWO attention groups
    concurrently (second group's ctx borrows the then-idle q-pass bank),
    alternating with out-proj units, hiding each group's exp->ctx chain
    latency in the other group's matmuls.
"""

import os
import numpy as np
import ml_dtypes
from collections import deque
from contextlib import ExitStack

DEBUG_DUMP = os.environ.get("BASSDEBUG", "") == "1"

import concourse.bass as bass
import concourse.bacc as bacc
import concourse.mybir as mybir
from concourse.tile import TileContext
from concourse.bass_utils import run_bass_kernel_spmd
from concourse.masks import make_identity

EMB, H, G, D, T = 4096, 32, 8, 128, 2048
EPS = 1e-6
NCORES = 8
HP = H // NCORES          # 4 query heads per core
NT = T // 128             # 16 t-tiles
NE = EMB // 128           # 32 e-tiles
NO = EMB // 512           # 8 output column tiles
QW = HP * D               # 512 = q width per core
KVW = 2 * D               # 256 = k|v width per core
SM_SCALE = 1.0 / float(np.sqrt(D))
NEG = -1e9
LA = 1                    # S-matmul lookahead within an attention group

F32 = mybir.dt.float32
BF16 = mybir.dt.bfloat16
BF = ml_dtypes.bfloat16

_prog_cache = {}


def _build_program():
    nc = bacc.Bacc()

    xT_d = nc.declare_dram_parameter("xT", [NT * 128, NE * 128], BF16, isOutput=False)
    wq_d = nc.declare_dram_parameter("wq", [128, NE * QW], BF16, isOutput=False)
    wkv_d = nc.declare_dram_parameter("wkv", [128, NE * KVW], BF16, isOutput=False)
    wo_d = nc.declare_dram_parameter("wo", [128, HP * EMB], BF16, isOutput=False)
    # [cosq x4 | sinq x4 | cosk | sink]; q tables tiled 4-wide to match the
    # half-split head-interleaved q layout (all first-halves, then second)
    cs_d = nc.declare_dram_parameter("cs", [NT * 128, 1280], F32, isOutput=False)
    mask_d = nc.declare_dram_parameter("maskT", [128, 512], F32, isOutput=False)
    bias_d = nc.declare_dram_parameter("biasb", [128, QW + KVW], F32, isOutput=False)
    out_d = nc.declare_dram_parameter("out", [T, EMB], BF16, isOutput=True)
    if DEBUG_DUMP:
        dbg_qT = nc.declare_dram_parameter("dbg_qT", [HP * 128, T], BF16,
                                           isOutput=True)
        dbg_kT = nc.declare_dram_parameter("dbg_kT", [128, T], BF16, isOutput=True)
        dbg_v = nc.declare_dram_parameter("dbg_v", [NT * 128, 128], BF16,
                                          isOutput=True)
        dbg_ctxT = nc.declare_dram_parameter("dbg_ctxT", [HP * 128, T], BF16,
                                             isOutput=True)
        dbg_pacc = nc.declare_dram_parameter("dbg_pacc", [16 * 128, 512], F32,
                                             isOutput=True)
        dbg_den = nc.declare_dram_parameter("dbg_den", [16 * 128, 512], F32,
                                            isOutput=True)
        dbg_ctxps = nc.declare_dram_parameter("dbg_ctxps", [16 * 128, 512], F32,
                                              isOutput=True)
        dbg_pt = nc.declare_dram_parameter("dbg_pt", [16 * 128, 512], BF16,
                                           isOutput=True)
        dbg_sps = nc.declare_dram_parameter("dbg_sps", [16 * 128, 512], BF16,
                                            isOutput=True)

    with TileContext(nc) as tc, ExitStack() as ctx:
        consts = ctx.enter_context(tc.tile_pool(name="consts", bufs=1))
        wpool = ctx.enter_context(tc.tile_pool(name="wpool", bufs=1))
        xpool = ctx.enter_context(
            tc.tile_pool(name="xpool", bufs=2 if DEBUG_DUMP else 3))
        cspool = ctx.enter_context(tc.tile_pool(name="cspool", bufs=2))
        scratch = ctx.enter_context(tc.tile_pool(name="scratch", bufs=2))
        rbpool = ctx.enter_context(tc.tile_pool(name="rbpool", bufs=6))
        small = ctx.enter_context(tc.tile_pool(name="small", bufs=4))
        ppool = ctx.enter_context(tc.tile_pool(name="ppool", bufs=4))
        epool = ctx.enter_context(tc.tile_pool(name="epool", bufs=2))
        opool = ctx.enter_context(tc.tile_pool(name="opool", bufs=2))
        resid = ctx.enter_context(tc.tile_pool(name="resid", bufs=1))
        pspool = ctx.enter_context(tc.tile_pool(name="ps", bufs=1, space="PSUM"))

        # ---- explicit PSUM placement (allocation order == address order) --
        # Rule: a start=True matmul marks its WHOLE 2KB bank pending-zero,
        # but Tile only orders instructions whose byte ranges overlap.  So
        # each bank's consecutive tenants must have overlapping ranges:
        #  - sring: S matmuls; write [q0:512], exp reads [q0:512] — and the
        #    same-bank reuse S(j+2) is always emitted AFTER exp(j).
        #  - mring: o_ps/den, all full [0:512] writers/readers.
        #  - ctx: single bank; group g+1's start overlaps group g's ctxT-mul
        #    read (both full range) so groups serialize automatically.
        #  - tp: one fixed [0:128] slot — every transpose/copy same range.
        sring = [pspool.tile([128, 512], F32, tag=f"sr{i}", name=f"sr{i}")
                 for i in range(3)]                          # banks 0-2
        mring = [pspool.tile([128, 512], F32, tag=f"mr{i}", name=f"mr{i}")
                 for i in range(2)]                          # banks 3-4
        q_ps_bank = pspool.tile([128, QW], F32, tag="qp0", name="qp0")  # bank 5
        ctx_bank = pspool.tile([128, 512], F32, tag="cx0", name="cx0")  # bank 6
        kv_ps = pspool.tile([128, KVW], F32, tag="kvp", name="kv_ps")  # bank 7

        _m_i = [0]

        def next_m():
            t = mring[_m_i[0] % 2]
            _m_i[0] += 1
            return t

        _s_i = [0]

        def next_s():
            t = sring[_s_i[0] % 3]
            _s_i[0] += 1
            return t

        # ---- weight/const SBUF tiles -------------------------------------
        wq_sb = wpool.tile([128, NE * QW], BF16, tag="wq", name="wq")
        wkv_sb = wpool.tile([128, NE * KVW], BF16, tag="wkv", name="wkv")
        wo_sb = wpool.tile([128, HP * EMB], BF16, tag="wo", name="wo")
        mask_sb = consts.tile([128, 512], F32, tag="mask", name="mask")
        bias_sb = consts.tile([128, QW + KVW], F32, tag="bias", name="bias")

        xstrips = {}
        css = {}

        def post_x(it):
            xs = xpool.tile([128, NE * 128], BF16, tag="xstrip", name=f"xstrip{it}")
            r0, r1 = it * 128, (it + 1) * 128
            nc.sync.dma_start(out=xs[:, 0:2048], in_=xT_d[r0:r1, 0:2048])
            nc.sync.dma_start(out=xs[:, 2048:4096], in_=xT_d[r0:r1, 2048:4096])
            xstrips[it] = xs

        def post_cs(it):
            cst = cspool.tile([128, 1280], F32, tag="cs", name=f"cs{it}")
            r0, r1 = it * 128, (it + 1) * 128
            nc.sync.dma_start(out=cst[:, 0:640], in_=cs_d[r0:r1, 0:640])
            nc.sync.dma_start(out=cst[:, 640:1280], in_=cs_d[r0:r1, 640:1280])
            css[it] = cst

        # ---- DMA posts in need-order -------------------------------------
        # First matmul needs wq chunk 0 + xstrip0; the fused strip0+1 loop
        # then consumes wq/wkv chunks in e-order at cold-clock pace.
        nc.sync.dma_start(out=wq_sb[:, 0:QW], in_=wq_d[:, 0:QW])
        post_x(0)
        nc.sync.dma_start(out=wq_sb[:, QW:4 * QW], in_=wq_d[:, QW:4 * QW])
        k0, k1 = 0, 4 * KVW
        nc.sync.dma_start(out=wkv_sb[:, k0:k1], in_=wkv_d[:, k0:k1])
        post_x(1)
        for ch in range(1, 8):
            c0, c1 = ch * 4 * QW, (ch + 1) * 4 * QW
            nc.sync.dma_start(out=wq_sb[:, c0:c1], in_=wq_d[:, c0:c1])
            k0, k1 = ch * 4 * KVW, (ch + 1) * 4 * KVW
            nc.sync.dma_start(out=wkv_sb[:, k0:k1], in_=wkv_d[:, k0:k1])
        nc.sync.dma_start(out=bias_sb, in_=bias_d[:, :])
        post_cs(0)
        post_cs(1)
        nc.sync.dma_start(out=mask_sb, in_=mask_d[:, :])
        post_x(2)
        for ch in range(4):
            c0, c1 = ch * HP * EMB // 4, (ch + 1) * HP * EMB // 4
            nc.sync.dma_start(out=wo_sb[:, c0:c1], in_=wo_d[:, c0:c1])

        # engine-side constants (no DMA involved)
        ident = consts.tile([128, 128], F32, tag="ident", name="ident")
        make_identity(nc, ident)
        ones_f32 = consts.tile([128, 128], F32, tag="ones_f32", name="ones_f32")
        nc.vector.memset(ones_f32, 1.0)
        eps_t = consts.tile([128, 1], F32, tag="eps", name="eps")
        nc.vector.memset(eps_t, EPS)

        # resident activations
        qT = [resid.tile([128, T], BF16, tag=f"qT{h}", name=f"qT{h}") for h in range(HP)]
        kT = resid.tile([128, T], BF16, tag="kT", name="kT")
        vsb = [resid.tile([128, 128], BF16, tag=f"v{j}", name=f"v{j}") for j in range(NT)]
        ctxT = [resid.tile([128, T], BF16, tag=f"ctxT{h}", name=f"ctxT{h}") for h in range(HP)]

        # ============== fill-queue machinery ==============================
        slot_i = [0]
        epiT_items = []          # list of (gate_slot, fn, strip)
        b_groups = deque()       # BGroup instances
        c_units = deque()        # closures
        groups_done = {}         # si -> count
        strips_T_done = [0]      # count of strips fully transposed
        toggle = [0]
        tail_mode = [False]      # after eloop15: run two B groups at once
        active_b = {}            # slot (0/1) -> running BGroup
        b_rr = [0]

        def strip_transposed(s):
            # strips complete in order; unlock B(si) when strip 4si+3 done
            if s % 4 == 3:
                si = s // 4
                for h in range(HP):
                    b_groups.append(BGroup(si, h))

        def group_finished(si):
            groups_done[si] = groups_done.get(si, 0) + 1
            if groups_done[si] == HP:
                push_c_units(si)

        def push_c_units(si):
            for it in range(4 * si, 4 * si + 4):
                holder = {}
                for ot in range(NO):
                    c_units.append(make_c_unit(it, ot, holder))

        def make_c_unit(it, ot, holder):
            def run():
                if ot == 0:
                    holder["osb"] = opool.tile([128, EMB], BF16, tag="osb",
                                               name="osb")
                osb = holder["osb"]
                o_ps = next_m()
                for hh in range(HP):
                    nc.tensor.matmul(
                        o_ps,
                        ctxT[hh][:, it * 128:(it + 1) * 128],
                        wo_sb[:, hh * EMB + ot * 512:hh * EMB + (ot + 1) * 512],
                        start=(hh == 0), stop=(hh == HP - 1),
                        skip_group_check=True,
                    )
                nc.scalar.copy(out=osb[:, ot * 512:(ot + 1) * 512], in_=o_ps)
                if ot == NO - 1:
                    nc.sync.dma_start(
                        out=out_d[it * 128:(it + 1) * 128, :], in_=osb)
            return run

        _group_ctr = [0]

        class BGroup:
            def __init__(self, si, h):
                self.si, self.h = si, h
                self.njb = 4 * si + 4
                self.j = 0
                self.ns = 0
                self.s_tiles = {}
                self.pts = {}
                self.ctx_slot = ctx_bank
                self.ctx_ps = None
                self.pacc = None
                self.gi = _group_ctr[0]
                _group_ctr[0] += 1

            def emit_S(self):
                jb = self.ns
                kk = jb - 4 * self.si
                q0 = 128 * kk if kk > 0 else 0
                s_ps = next_s()
                nc.tensor.matmul(
                    s_ps[:, q0:512], kT[:, jb * 128:(jb + 1) * 128],
                    qT[self.h][:, self.si * 512 + q0:(self.si + 1) * 512],
                    start=True, stop=True, skip_group_check=True,
                )
                if kk >= 0:  # diagonal (partially masked) block
                    nc.vector.tensor_add(
                        s_ps[:, q0:512], s_ps[:, q0:512],
                        mask_sb[:, 0:512 - q0])
                if DEBUG_DUMP and self.gi == 12:
                    ssb = epool.tile([128, 512], BF16, tag="dbg3", name="dbg3", bufs=1)
                    nc.vector.tensor_copy(out=ssb[:, q0:512],
                                          in_=s_ps[:, q0:512])
                    nc.sync.dma_start(
                        out=dbg_sps[jb * 128:(jb + 1) * 128, q0:512],
                        in_=ssb[:, q0:512])
                self.s_tiles[jb] = (s_ps, q0)
                self.ns += 1

            def emit_exp(self, jb):
                s_ps, q0 = self.s_tiles.pop(jb)
                p_t = ppool.tile([128, 512], BF16, tag="pt", name="pt")
                nc.scalar.activation(
                    out=p_t[:, q0:512], in_=s_ps[:, q0:512],
                    func=mybir.ActivationFunctionType.Exp,
                    scale=SM_SCALE,
                )
                if DEBUG_DUMP and self.gi == 12:
                    nc.sync.dma_start(
                        out=dbg_pt[jb * 128:(jb + 1) * 128, q0:512],
                        in_=p_t[:, q0:512])
                if jb == 0:
                    self.ctx_ps = self.ctx_slot
                    self.pacc = epool.tile([128, 512], F32, tag="pacc",
                                           name="pacc")
                    nc.vector.tensor_copy(out=self.pacc, in_=p_t)
                else:
                    nc.vector.tensor_add(self.pacc[:, q0:512],
                                         self.pacc[:, q0:512], p_t[:, q0:512])
                self.pts[jb] = (p_t, q0)

            def emit_next(self):
                """One pipeline pop; return False when exhausted.  pop(0):
                S-burst + exp(0).  pop(j>=1): exp(j) + S(j+1) + pacc(j),
                then ctx(j-1) — every ctx has a full pop (plus the other
                group's pops and C units, in the tail) behind its exp."""
                si, h, njb = self.si, self.h, self.njb
                jp = self.j
                if jp == 0:
                    while self.ns < min(2, njb):
                        self.emit_S()
                    self.emit_exp(0)
                    self.j += 1
                    return True
                if jp < njb:
                    self.emit_exp(jp)
                    # one new S per pop, AFTER this pop's exp (zero-region:
                    # its ring slot's previous exp is already emitted)
                    if self.ns < njb:
                        self.emit_S()
                jb = jp - 1
                p_t, q0 = self.pts.pop(jb)
                nc.tensor.matmul(self.ctx_ps[:, q0:512], vsb[jb],
                                 p_t[:, q0:512],
                                 start=(jb == 0), stop=(jb == njb - 1),
                                 skip_group_check=True)
                if jb == njb - 1:
                    den_ps = next_m()
                    nc.tensor.matmul(den_ps, ones_f32, self.pacc,
                                     start=True, stop=True,
                                     skip_group_check=True)
                    rden = epool.tile([128, 512], F32, tag="rden",
                                      name="rden")
                    nc.vector.reciprocal_approx_fast(out=rden, in_=den_ps)
                    if DEBUG_DUMP:
                        g0 = self.gi * 128
                        nc.sync.dma_start(out=dbg_pacc[g0:g0 + 128, :],
                                          in_=self.pacc)
                        dsb = epool.tile([128, 512], F32, tag="dbg",
                                         name="dbg", bufs=1)
                        nc.scalar.copy(out=dsb, in_=den_ps)
                        nc.sync.dma_start(out=dbg_den[g0:g0 + 128, :],
                                          in_=dsb)
                        dsb2 = epool.tile([128, 512], F32, tag="dbg2",
                                          name="dbg2", bufs=1)
                        nc.scalar.copy(out=dsb2, in_=self.ctx_ps)
                        nc.sync.dma_start(out=dbg_ctxps[g0:g0 + 128, :],
                                          in_=dsb2)
                    nc.vector.tensor_mul(
                        ctxT[h][:, si * 512:(si + 1) * 512],
                        self.ctx_ps, rden)
                self.j += 1
                return self.j <= njb

        def pe_slot():
            slot_i[0] += 1
            # delayed transposes first (they unlock attention groups)
            for idx, (gate, fn, s) in enumerate(epiT_items):
                if gate <= slot_i[0]:
                    epiT_items.pop(idx)
                    fn()
                    return
                break  # strictly in-order
            # alternate attention blocks and out-proj units
            first_b = toggle[0] == 0
            toggle[0] ^= 1
            for pick in ((0, 1) if first_b else (1, 0)):
                if pick == 0:
                    nslots = 2 if tail_mode[0] else 1
                    for sl in range(nslots):
                        if active_b.get(sl) is None and b_groups:
                            g = b_groups.popleft()
                            g.ctx_slot = ctx_bank if sl == 0 else q_ps_bank
                            active_b[sl] = g
                    for k in range(2):
                        sl = (b_rr[0] + k) % 2
                        g = active_b.get(sl)
                        if g is not None:
                            b_rr[0] = (sl + 1) % 2
                            if not g.emit_next():
                                active_b[sl] = None
                                group_finished(g.si)
                            return
                if pick == 1 and c_units:
                    c_units.popleft()()
                    return

        # ============== strip epilogue (DVE part + delayed PE part) =======
        def epi_dve(it, gate_delay=12):
            """bias-adds already ran (qsb/kvsb hold q|k|v + bias in f32).
            Emits the norm/rope DVE+ACT chain; pushes 5 transpose items."""
            qsb, kvsb, cst = strip_io[it]
            q3d = qsb.rearrange("p (two h d) -> p two h d", two=2, h=HP, d=64)
            sq5 = small.tile([128, 8], F32, tag="sq5", name="sq5")
            for b in range(HP + 1):
                src = q3d[:, :, b] if b < HP else kvsb[:, 0:128]
                sqout = scratch.tile([128, 128], F32, tag="sqout", name="sqout")
                so = (sqout.rearrange("p (two d) -> p two d", two=2)
                      if b < HP else sqout)
                nc.scalar.activation(
                    out=so, in_=src,
                    func=mybir.ActivationFunctionType.Square,
                    accum_out=sq5[:, b:b + 1],
                )
            rstd5 = small.tile([128, 8], F32, tag="rstd5", name="rstd5")
            nc.scalar.activation(
                out=rstd5[:, 0:5], in_=sq5[:, 0:5],
                func=mybir.ActivationFunctionType.Sqrt,
                bias=eps_t, scale=1.0 / D,
            )
            nc.vector.reciprocal_approx_fast(out=rstd5[:, 0:5], in_=rstd5[:, 0:5])
            # q rope, all 4 heads at once: out1 = x1*c1 - x2*s1; out2 = x2*c2 + x1*s2
            x1, x2 = qsb[:, 0:256], qsb[:, 256:512]
            qrt = scratch.tile([128, 512], F32, tag="qrt", name="qrt")
            qm = scratch.tile([128, 256], F32, tag="qm", name="qm")
            nc.vector.tensor_mul(qrt[:, 0:256], x1, cst[:, 0:256])
            nc.vector.tensor_mul(qm, x2, cst[:, 512:768])
            nc.vector.tensor_sub(qrt[:, 0:256], qrt[:, 0:256], qm)
            nc.vector.tensor_mul(qrt[:, 256:512], x2, cst[:, 256:512])
            nc.vector.tensor_mul(qm, x1, cst[:, 768:1024])
            nc.vector.tensor_add(qrt[:, 256:512], qrt[:, 256:512], qm)
            qrt3d = qrt.rearrange("p (two h d) -> p two h d", two=2, h=HP, d=64)
            # k rope
            ksrc = kvsb[:, 0:128]
            c_t, s_t = cst[:, 1024:1152], cst[:, 1152:1280]
            krt = scratch.tile([128, 128], F32, tag="krt", name="krt")
            km = scratch.tile([128, 64], F32, tag="km", name="km")
            nc.vector.tensor_mul(krt[:, 0:64], ksrc[:, 0:64], c_t[:, 0:64])
            nc.vector.tensor_mul(km, ksrc[:, 64:128], s_t[:, 0:64])
            nc.vector.tensor_sub(krt[:, 0:64], krt[:, 0:64], km)
            nc.vector.tensor_mul(krt[:, 64:128], ksrc[:, 64:128], c_t[:, 64:128])
            nc.vector.tensor_mul(km, ksrc[:, 0:64], s_t[:, 64:128])
            nc.vector.tensor_add(krt[:, 64:128], krt[:, 64:128], km)
            rbs = []
            for b in range(HP + 1):  # 0..3 q heads, 4 = k
                rb = rbpool.tile([128, 128], F32, tag="rb", name="rb")
                if b < HP:
                    nc.vector.tensor_scalar_mul(
                        rb.rearrange("p (two d) -> p two d", two=2),
                        qrt3d[:, :, b], rstd5[:, b:b + 1])
                else:
                    nc.vector.tensor_scalar_mul(rb, krt, rstd5[:, b:b + 1])
                rbs.append(rb)
            # v copy to resident (bf16 cast)
            nc.scalar.copy(out=vsb[it], in_=kvsb[:, 128:256])

            def make_T(b, rb):
                def run():
                    tp = next_m()[:, 0:128]
                    nc.tensor.transpose(tp, rb, ident)
                    dst = qT[b] if b < HP else kT
                    nc.scalar.copy(out=dst[:, it * 128:(it + 1) * 128], in_=tp)
                    if b == HP:
                        strips_T_done[0] += 1
                        strip_transposed(it)
                return run

            base = slot_i[0] + gate_delay
            for b in range(HP + 1):
                epiT_items.append((base + 2 * b, make_T(b, rbs[b]), it))

        strip_io = {}

        def bias_q(it, q_ps_t):
            qsb = scratch.tile([128, QW], F32, tag="qsb", name=f"qsb{it}")
            nc.vector.tensor_add(qsb, q_ps_t, bias_sb[:, 0:QW])
            strip_io.setdefault(it, [None, None, None])[0] = qsb

        def bias_kv(it, kv_ps_t):
            kvsb = scratch.tile([128, KVW], F32, tag="kvsb", name=f"kvsb{it}")
            nc.vector.tensor_add(kvsb, kv_ps_t, bias_sb[:, QW:QW + KVW])
            strip_io[it][1] = kvsb
            strip_io[it][2] = css[it]

        # ============== phase A ===========================================
        # strips 0+1 share one e-loop so the PE consumes freshly arriving
        # weight chunks at half rate during the initial weight download
        q01 = [sring[0], sring[1]]
        # pack kv0/kv1: kv0 -> the kv slot, kv1 borrows mring[0][:, 0:256]
        kv01 = [kv_ps, mring[0][:, 0:KVW]]
        for e in range(NE):
            for it in range(2):
                xt = xstrips[it][:, e * 128:(e + 1) * 128]
                nc.tensor.matmul(q01[it], xt, wq_sb[:, e * QW:(e + 1) * QW],
                                 start=(e == 0), stop=(e == NE - 1),
                                 skip_group_check=True)
                nc.tensor.matmul(kv01[it], xt, wkv_sb[:, e * KVW:(e + 1) * KVW],
                                 start=(e == 0), stop=(e == NE - 1),
                                 skip_group_check=True)
        bias_q(0, q01[0])
        bias_q(1, q01[1])
        bias_kv(0, kv01[0])
        bias_kv(1, kv01[1])
        epi_dve(0, gate_delay=12)
        epi_dve(1, gate_delay=36)

        for it in range(2, NT):
            if it + 1 < NT:
                post_x(it + 1)
            post_cs(it)
            q_ps_t = q_ps_bank
            xstrip = xstrips[it]
            qmod, kvmod = 2, 4
            # q-pass
            for e in range(NE):
                xt = xstrip[:, e * 128:(e + 1) * 128]
                nc.tensor.matmul(q_ps_t, xt, wq_sb[:, e * QW:(e + 1) * QW],
                                 start=(e == 0), stop=(e == NE - 1),
                                 skip_group_check=True)
                if e % qmod == qmod - 1:
                    pe_slot()
            bias_q(it, q_ps_t)
            # kv-pass
            for e in range(NE):
                xt = xstrip[:, e * 128:(e + 1) * 128]
                nc.tensor.matmul(kv_ps, xt, wkv_sb[:, e * KVW:(e + 1) * KVW],
                                 start=(e == 0), stop=(e == NE - 1),
                                 skip_group_check=True)
                if e % kvmod == kvmod - 1:
                    pe_slot()
            bias_kv(it, kv_ps)
            if it >= 3:
                epi_dve(it - 1)

        epi_dve(NT - 1, gate_delay=8)
        tail_mode[0] = True

        # ============== tail: drain remaining attention + out-proj ========
        guard = 0
        while epiT_items or b_groups or c_units or any(
                g is not None for g in active_b.values()):
            pe_slot()
            guard += 1
            assert guard < 5000, "fill queue drain stuck"

        if DEBUG_DUMP:
            for h in range(HP):
                nc.sync.dma_start(out=dbg_qT[h * 128:(h + 1) * 128, :], in_=qT[h])
                nc.sync.dma_start(out=dbg_ctxT[h * 128:(h + 1) * 128, :],
                                  in_=ctxT[h])
            nc.sync.dma_start(out=dbg_kT[:, :], in_=kT)
            for j in range(NT):
                nc.sync.dma_start(out=dbg_v[j * 128:(j + 1) * 128, :], in_=vsb[j])

    return nc


def _prep_inputs(x, mask, cos, sin, wq, bq, wk, bk, wv, bv, wo, q_scale, k_scale):
    x2 = np.asarray(x, dtype=np.float32).reshape(T, EMB)
    # strip layout: row (it*128 + p), col (eb*128 + t) holds x[it*128+t, eb*128+p]
    xTt = x2.reshape(NT, 128, NE, 128).transpose(0, 3, 2, 1)
    xTt = np.ascontiguousarray(xTt).reshape(NT * 128, NE * 128).astype(BF)

    qs = np.asarray(q_scale, dtype=np.float32)
    ks = np.asarray(k_scale, dtype=np.float32)
    qs_rot = np.concatenate([qs[64:], qs[:64]])
    ks_rot = np.concatenate([ks[64:], ks[:64]])
    cos = np.asarray(cos, dtype=np.float32)
    sin = np.asarray(sin, dtype=np.float32)
    cosq = cos * qs[None, :]
    sinq = sin * qs_rot[None, :]
    # q tables tiled 4-wide: [c1 x4 | c2 x4], matching half-split q layout
    cq4 = np.concatenate([np.tile(cosq[:, 0:64], (1, HP)),
                          np.tile(cosq[:, 64:128], (1, HP))], axis=1)
    sq4 = np.concatenate([np.tile(sinq[:, 0:64], (1, HP)),
                          np.tile(sinq[:, 64:128], (1, HP))], axis=1)
    cs = np.concatenate([cq4, sq4, cos * ks[None, :], sin * ks_rot[None, :]],
                        axis=1)
    cs = np.ascontiguousarray(cs, dtype=np.float32)

    def q_halfsplit(a):
        # permute last axis from [h][half][d'] to [half][h][d']
        return (a.reshape(*a.shape[:-1], HP, 2, 64)
                .swapaxes(-3, -2)
                .reshape(*a.shape))

    jj = np.arange(128)[:, None]
    cc = np.arange(512)[None, :]
    maskT = np.where(jj > cc, NEG, 0.0).astype(np.float32)

    wq = np.asarray(wq, dtype=np.float32)
    wk = np.asarray(wk, dtype=np.float32)
    wv = np.asarray(wv, dtype=np.float32)
    wo = np.asarray(wo, dtype=np.float32)
    bq = np.asarray(bq, dtype=np.float32)
    bk = np.asarray(bk, dtype=np.float32)
    bv = np.asarray(bv, dtype=np.float32)

    in_maps = []
    for c in range(NCORES):
        # [p, e*QW + o] = wq[e*128 + p, c*QW + perm(o)]
        wq_c = q_halfsplit(wq[:, c * QW:(c + 1) * QW]).reshape(NE, 128, QW)
        wq_c = np.ascontiguousarray(wq_c.transpose(1, 0, 2)).reshape(128, NE * QW)
        wkv_c = np.concatenate(
            [wk[:, c * D:(c + 1) * D], wv[:, c * D:(c + 1) * D]], axis=1)
        wkv_c = wkv_c.reshape(NE, 128, KVW)
        wkv_c = np.ascontiguousarray(wkv_c.transpose(1, 0, 2)).reshape(128, NE * KVW)
        # [p, h*EMB + col] = wo[c*QW + h*128 + p, col]
        wo_c = wo[c * QW:(c + 1) * QW, :].reshape(HP, 128, EMB)
        wo_c = np.ascontiguousarray(wo_c.transpose(1, 0, 2)).reshape(128, HP * EMB)
        bias_c = np.broadcast_to(
            np.concatenate([q_halfsplit(bq[c * QW:(c + 1) * QW]),
                            bk[c * D:(c + 1) * D], bv[c * D:(c + 1) * D]]),
            (128, QW + KVW))
        in_maps.append({
            "xT": xTt,
            "wq": wq_c.astype(BF),
            "wkv": wkv_c.astype(BF),
            "wo": wo_c.astype(BF),
            "cs": cs,
            "maskT": maskT,
            "biasb": np.ascontiguousarray(bias_c, dtype=np.float32),
        })
    return in_maps


def _get_program():
    if "nc" not in _prog_cache:
        nc = _build_program()
        if not nc.is_finalized():
            nc.finalize()
        _prog_cache["nc"] = nc
    return _prog_cache["nc"]


def kernel(**inputs):
    in_maps = _prep_inputs(**inputs)
    nc = _get_program()
    res = run_bass_kernel_spmd(nc, in_maps, list(range(NCORES)))
    out = np.zeros((T, EMB), dtype=np.float32)
    for r in res.results:
        out += np.asarray(r["out"], dtype=np.float32)
    return out.reshape(1, T, EMB)
